# revision 1
# baseline (speedup 1.0000x reference)
"""Channel attention (B=8, N=16384, C=512) Trainium2 Bass kernel.

Math (per batch b, with v = x^T [C, N]):
    energy  = v @ v^T                      [C, C]   (gram matrix, symmetric)
    att     = softmax(rowmax(e) - e)       == exp(rowmin(e) - e) / Z  (shift-invariant)
    out     = gamma * (att @ v) + v        [C, N]
    y       = out^T                        [N, C]

Sharding: data-parallel over B — core b computes batch b entirely.

Per-core dataflow:
  Phase 1 (stream x in 128-row chunks):
    hi = fp16(x), lo = fp16(x - hi)  (split-precision so the energy matrix is
    accurate to ~1e-4 absolute — the softmax exponent is sensitive)
    energy ~= Hi^T Hi + Hi^T Lo + Lo^T Hi accumulated in PSUM fp32.
    Only the upper block-triangle is computed (energy is symmetric).
    hi is also written to a DRAM scratch for the later transposed read.
  Interlude:
    mirror the triangle via PE transposes; rowmin; exp(min - e) with fused
    row-sum (ACT accum_out); W = I + gamma/Z * att^T in fp16 via PE transposes.
    (W folds the softmax normalization, the gamma scale AND the residual.)
  Phase 2:
    hiT = DMA-xbar-transposed load of hi  [C, N] fp16 (resident in SBUF)
    y[n-chunk] = hiT-slice^T @ W  accumulated over 4 channel blocks
    (= x @ (I + gamma*att^T) = gamma*(att@v)^T + x, i.e. the final output).
"""

import sys

sys.path.insert(0, "/opt/trn_rl_repo")

from contextlib import ExitStack

import numpy as np

import concourse.bass as bass
import concourse.mybir as mybir
import concourse.tile as tile
from concourse import bacc
from concourse.bass_utils import run_bass_kernel_spmd
from concourse.masks import make_identity

B, N, C = 8, 16384, 512
P = 128
NK = N // P  # 128 row chunks
NB = C // P  # 4 channel blocks
F32 = mybir.dt.float32
F16 = mybir.dt.float16

_nc_cache = None


def _build():
    nc = bacc.Bacc()
    x_in = nc.dram_tensor("x", [N, C], F32, kind="ExternalInput")
    g_in = nc.dram_tensor("gamma", [1], F32, kind="ExternalInput")
    y_out = nc.dram_tensor("y", [N, C], F32, kind="ExternalOutput")
    NQ = 4  # DRAM scratch split per quarter for precise DMA dependency tracking
    hi_dram = [nc.dram_tensor(f"hi_scratch{q}", [N // NQ, C], F16) for q in range(NQ)]

    with ExitStack() as ctx:
        tc = ctx.enter_context(tile.TileContext(nc))
        const = ctx.enter_context(tc.tile_pool(name="const", bufs=1))
        xpool = ctx.enter_context(tc.tile_pool(name="xpool", bufs=2))
        hpool = ctx.enter_context(tc.tile_pool(name="hpool", bufs=2))
        lpool = ctx.enter_context(tc.tile_pool(name="lpool", bufs=2))
        soft = ctx.enter_context(tc.tile_pool(name="soft", bufs=1))
        hiT_pool = ctx.enter_context(tc.tile_pool(name="hiT", bufs=1))
        opool = ctx.enter_context(tc.tile_pool(name="opool", bufs=2))
        psum_e_ctx = tc.tile_pool(name="psum_e", bufs=1, space="PSUM")
        psum_e = psum_e_ctx.__enter__()

        ident16 = const.tile([P, P], F16)
        make_identity(nc, ident16)
        ident32 = const.tile([P, P], F32)
        make_identity(nc, ident32)
        gamma_sb = const.tile([P, 1], F32)
        nc.sync.dma_start(out=gamma_sb, in_=g_in[:].to_broadcast([P, 1]))

        x_ap = x_in[:]
        # [P, NK, C] views: partition = row-within-chunk, mid = chunk index
        x_v = x_ap.rearrange("(n p) c -> p n c", p=P)
        hi_v = [h[:].rearrange("(n p) c -> p n c", p=P) for h in hi_dram]
        y_v = y_out[:].rearrange("(n p) c -> p n c", p=P)
        KB = 4  # k-chunks per phase-1 iteration
        CB = 4  # n-chunks per phase-2 iteration
        QC = NK // NQ  # chunks per quarter (32)

        # upper-triangle energy accumulators: row-block bi holds cols [bi*P, C)
        e_ps = [psum_e.tile([P, C - bi * P], F32, name=f"e{bi}", tag=f"e{bi}", bufs=1) for bi in range(NB)]
        # resident transposed hi [C, N] as 4 partition-blocks
        hiT = [hiT_pool.tile([P, N], F16, name=f"hiT{bj}", tag=f"hiT{bj}") for bj in range(NB)]

        # ---------------- Phase 1: energy (and hi/lo generation) ----------------
        for kb in range(NK // KB):
            k0 = kb * KB
            xk = xpool.tile([P, KB, C], F32)
            nc.sync.dma_start(out=xk, in_=x_v[:, k0 : k0 + KB, :])
            hk = hpool.tile([P, KB, C], F16)
            nc.scalar.copy(out=hk, in_=xk)  # fp16 round
            lk = lpool.tile([P, KB, C], F16)
            nc.vector.tensor_sub(lk, xk, hk)  # fp16(x - hi), mixed-dtype TT
            q_now = k0 // QC
            nc.sync.dma_start(out=hi_v[q_now][:, k0 % QC : k0 % QC + KB, :], in_=hk)

            for u in range(KB):
                k = k0 + u
                first = k == 0
                last = k == NK - 1
                for bi in range(NB):
                    j0 = bi * P
                    lhs_h = hk[:, u, j0 : j0 + P]
                    lhs_l = lk[:, u, j0 : j0 + P]
                    rhs_h = hk[:, u, j0:C]
                    rhs_l = lk[:, u, j0:C]
                    nc.tensor.matmul(e_ps[bi], lhs_h, rhs_h, start=first, stop=False)
                    nc.tensor.matmul(e_ps[bi], lhs_h, rhs_l, start=False, stop=False)
                    nc.tensor.matmul(e_ps[bi], lhs_l, rhs_h, start=False, stop=last)

            # transposed hi reads for quarter q, spread over quarter q+1's
            # iterations (deps on hi_dram[q] are long satisfied), alternating
            # HWDGE rings so neither queue bursts
            qprev = k0 // QC - 1
            step = (k0 % QC) // KB  # 0..7 within the quarter
            if qprev >= 0 and step % 2 == 0:
                bj = step // 2
                nc.scalar.dma_start_transpose(
                    out=hiT[bj][:, qprev * (N // NQ) : (qprev + 1) * (N // NQ)],
                    in_=hi_dram[qprev][:][:, bj * P : (bj + 1) * P],
                )

        n0 = 3 * (N // NQ)
        for bj in range(NB):
            nc.sync.dma_start_transpose(
                out=hiT[bj][:, n0 : n0 + N // NQ],
                in_=hi_dram[3][:][:, bj * P : (bj + 1) * P],
            )

        # ---------------- Interlude: softmax -> W = I + gamma * att^T ----------------
        e_row = [soft.tile([P, C], F32, name=f"erow{bi}", tag=f"erow{bi}") for bi in range(NB)]
        for bi in range(NB):
            nc.scalar.copy(out=e_row[bi][:, bi * P : C], in_=e_ps[bi])
        psum_e_ctx.__exit__(None, None, None)
        psum_t_ctx = tc.tile_pool(name="psum_t", bufs=2, space="PSUM")
        psum_t = psum_t_ctx.__enter__()
        # mirror the strict-lower blocks from the stored upper triangle
        for bi in range(NB):
            for bj in range(bi):
                pt = psum_t.tile([P, P], F32, tag="tp")
                nc.tensor.transpose(pt, e_row[bj][:, bi * P : (bi + 1) * P], ident32)
                nc.scalar.copy(out=e_row[bi][:, bj * P : (bj + 1) * P], in_=pt)

        W = [soft.tile([P, C], F16, name=f"W{bj}", tag=f"W{bj}") for bj in range(NB)]
        Bp = [soft.tile([P, C], F16, name=f"Bp{bi}", tag=f"Bp{bi}") for bi in range(NB)]
        for bi in range(NB):
            mn = soft.tile([P, 1], F32, tag=f"mn{bi}")
            nc.vector.tensor_reduce(
                out=mn, in_=e_row[bi], axis=mybir.AxisListType.X, op=mybir.AluOpType.min
            )
            bt = soft.tile([P, C], F32, tag=f"bt{bi}")
            zt = soft.tile([P, 1], F32, tag=f"zt{bi}")
            nc.scalar.activation(
                out=bt,
                in_=e_row[bi],
                func=mybir.ActivationFunctionType.Exp,
                bias=mn,
                scale=-1.0,
                accum_out=zt,
            )
            rz = soft.tile([P, 1], F32, tag=f"rz{bi}")
            nc.vector.reciprocal(out=rz, in_=zt)
            gr = soft.tile([P, 1], F32, tag=f"gr{bi}")
            nc.vector.tensor_mul(gr, rz, gamma_sb)
            nc.vector.tensor_scalar_mul(Bp[bi], bt, gr)  # fp16: gamma*att rows
        for bi in range(NB):
            for bj in range(NB):
                pt = psum_t.tile([P, P], F16, name="pt16", tag="tp16")
                nc.tensor.transpose(pt, Bp[bi][:, bj * P : (bj + 1) * P], ident16)
                nc.scalar.copy(out=W[bj][:, bi * P : (bi + 1) * P], in_=pt)
        for bj in range(NB):
            nc.vector.tensor_add(
                W[bj][:, bj * P : (bj + 1) * P], W[bj][:, bj * P : (bj + 1) * P], ident16
            )

        psum_t_ctx.__exit__(None, None, None)
        psum = ctx.enter_context(tc.tile_pool(name="psum", bufs=2, space="PSUM"))

        # ---------------- Phase 2: y = x @ W ----------------
        for cb in range(NK // CB):
            c0 = cb * CB
            ops = psum.tile([P, CB, C], F32, tag="ops", bufs=2)
            for u in range(CB):
                r0 = (c0 + u) * P
                for bj in range(NB):
                    nc.tensor.matmul(
                        ops[:, u, :],
                        hiT[bj][:, r0 : r0 + P],
                        W[bj],
                        start=(bj == 0),
                        stop=(bj == NB - 1),
                    )
            ob = opool.tile([P, CB, C], F32)
            nc.scalar.copy(out=ob, in_=ops)
            nc.sync.dma_start(out=y_v[:, c0 : c0 + CB, :], in_=ob)

    nc.finalize()
    return nc


def _get_nc():
    global _nc_cache
    if _nc_cache is None:
        _nc_cache = _build()
    return _nc_cache


def kernel(x, gamma, _trace=False):
    x = np.ascontiguousarray(np.asarray(x), dtype=np.float32)
    gamma = np.ascontiguousarray(np.asarray(gamma), dtype=np.float32)
    nc = _get_nc()
    in_maps = [
        {"x": np.ascontiguousarray(x[b]), "gamma": gamma} for b in range(B)
    ]
    res = run_bass_kernel_spmd(nc, in_maps, list(range(B)), trace=_trace)
    out = np.stack([r["y"] for r in res.results], axis=0)
    if _trace:
        return out, res
    return out



# revision 3
# speedup vs baseline: 1.5066x; 1.5066x over previous
"""Channel attention (B=8, N=16384, C=512) Trainium2 Bass kernel.

Math (per batch b, with v = x^T [C, N]):
    energy  = v @ v^T                      [C, C]   (gram matrix, symmetric)
    att     = softmax(rowmax(e) - e)       == exp(rowmin(e) - e) / Z  (shift-invariant)
    out     = gamma * (att @ v) + v        [C, N]
    y       = out^T                        [N, C]

Sharding: data-parallel over B — core b computes batch b entirely.

Per-core dataflow (single fp16 pass; the 2e-2 rel-err budget has ~30x slack):
  Phase 1 (stream x in 128-row chunks):
    hk = fp16(x)  (ACT)
    energy += hk-block^T @ hk-rowslice  in PSUM fp32, upper block-triangle only
    v-blocks = PE-transpose(hk blocks) -> PSUM fp16 -> DVE copy -> resident
    SBUF v [C, N] fp16 (no DRAM round-trip for the transposed read).
  Interlude:
    mirror the triangle via PE transposes; rowmin; exp(min - e) with fused
    row-sum (ACT accum_out); W = I + gamma/Z * att^T in fp16 via PE transposes.
    (W folds the softmax normalization, the gamma scale AND the residual.)
  Phase 2:
    y[n-chunk] = v-slice^T @ W  accumulated over 4 channel blocks
    (= x @ (I + gamma*att^T) = gamma*(att@v)^T + x, i.e. the final output).
"""

import sys

sys.path.insert(0, "/opt/trn_rl_repo")

from contextlib import ExitStack

import numpy as np

import concourse.bass as bass
import concourse.mybir as mybir
import concourse.tile as tile
from concourse import bacc
from concourse.bass_utils import run_bass_kernel_spmd
from concourse.masks import make_identity

B, N, C = 8, 16384, 512
P = 128
NK = N // P  # 128 row chunks
NB = C // P  # 4 channel blocks
KB = 4  # chunks per phase-1 iteration
CB = 4  # chunks per phase-2 iteration
F32 = mybir.dt.float32
F16 = mybir.dt.float16

_nc_cache = None


def _build():
    nc = bacc.Bacc()
    x_in = nc.dram_tensor("x", [N, C], F32, kind="ExternalInput")
    g_in = nc.dram_tensor("gamma", [1], F32, kind="ExternalInput")
    y_out = nc.dram_tensor("y", [N, C], F32, kind="ExternalOutput")

    with ExitStack() as ctx:
        tc = ctx.enter_context(tile.TileContext(nc))
        const = ctx.enter_context(tc.tile_pool(name="const", bufs=1))
        xpool = ctx.enter_context(tc.tile_pool(name="xpool", bufs=2))
        hpool = ctx.enter_context(tc.tile_pool(name="hpool", bufs=2))
        vpool = ctx.enter_context(tc.tile_pool(name="vpool", bufs=1))
        soft = ctx.enter_context(tc.tile_pool(name="soft", bufs=1))
        opool = ctx.enter_context(tc.tile_pool(name="opool", bufs=2))
        psum_e_ctx = tc.tile_pool(name="psum_e", bufs=1, space="PSUM")
        psum_e = psum_e_ctx.__enter__()
        psum_tp_ctx = tc.tile_pool(name="psum_tp", bufs=2, space="PSUM")
        psum_tp = psum_tp_ctx.__enter__()

        ident16 = const.tile([P, P], F16)
        make_identity(nc, ident16)
        ident32 = const.tile([P, P], F32)
        make_identity(nc, ident32)
        gamma_sb = const.tile([P, 1], F32)
        nc.sync.dma_start(out=gamma_sb, in_=g_in[:].to_broadcast([P, 1]))

        # [P, NK, C] views: partition = row-within-chunk, mid = chunk index
        x_v = x_in[:].rearrange("(n p) c -> p n c", p=P)
        y_v = y_out[:].rearrange("(n p) c -> p n c", p=P)

        # upper-triangle energy accumulators: row-block bi holds cols [bi*P, C)
        e_ps = [psum_e.tile([P, C - bi * P], F32, name=f"e{bi}", tag=f"e{bi}", bufs=1) for bi in range(NB)]
        # resident transposed x [C, N] fp16 as 4 partition-blocks
        v_sb = [vpool.tile([P, N], F16, name=f"v{bj}", tag=f"v{bj}") for bj in range(NB)]

        # ---------------- Phase 1: energy + transposed copy ----------------
        for kb in range(NK // KB):
            k0 = kb * KB
            xk = xpool.tile([P, KB, C], F32)
            nc.sync.dma_start(out=xk, in_=x_v[:, k0 : k0 + KB, :])
            hk = hpool.tile([P, KB, C], F16)
            nc.scalar.copy(out=hk, in_=xk)  # fp16 round
            pt = psum_tp.tile([P, KB, C], F16, tag="pt", bufs=2)
            for u in range(KB):
                k = k0 + u
                first = k == 0
                last = k == NK - 1
                for bi in range(NB):
                    j0 = bi * P
                    nc.tensor.matmul(
                        e_ps[bi], hk[:, u, j0 : j0 + P], hk[:, u, j0:C], start=first, stop=last
                    )
                for bj in range(NB):
                    nc.tensor.transpose(
                        pt[:, u, bj * P : (bj + 1) * P], hk[:, u, bj * P : (bj + 1) * P], ident16
                    )
            for bj in range(NB):
                nc.vector.tensor_copy(
                    out=v_sb[bj][:, k0 * P : (k0 + KB) * P], in_=pt[:, :, bj * P : (bj + 1) * P]
                )

        # ---------------- Interlude: softmax -> W = I + gamma * att^T ----------------
        e_row = [soft.tile([P, C], F32, name=f"erow{bi}", tag=f"erow{bi}") for bi in range(NB)]
        for bi in range(NB):
            nc.scalar.copy(out=e_row[bi][:, bi * P : C], in_=e_ps[bi])
        psum_tp_ctx.__exit__(None, None, None)
        psum_e_ctx.__exit__(None, None, None)
        psum_i_ctx = tc.tile_pool(name="psum_i", bufs=2, space="PSUM")
        psum_i = psum_i_ctx.__enter__()
        # mirror the strict-lower blocks from the stored upper triangle
        for bi in range(NB):
            for bj in range(bi):
                ptm = psum_i.tile([P, P], F32, tag="tp")
                nc.tensor.transpose(ptm, e_row[bj][:, bi * P : (bi + 1) * P], ident32)
                nc.vector.tensor_copy(out=e_row[bi][:, bj * P : (bj + 1) * P], in_=ptm)

        W = [soft.tile([P, C], F16, name=f"W{bj}", tag=f"W{bj}") for bj in range(NB)]
        for bi in range(NB):
            mn = soft.tile([P, 1], F32, tag=f"mn{bi}")
            nc.vector.tensor_reduce(
                out=mn, in_=e_row[bi], axis=mybir.AxisListType.X, op=mybir.AluOpType.min
            )
            bt = soft.tile([P, C], F32, tag=f"bt{bi}")
            zt = soft.tile([P, 1], F32, tag=f"zt{bi}")
            nc.scalar.activation(
                out=bt,
                in_=e_row[bi],
                func=mybir.ActivationFunctionType.Exp,
                bias=mn,
                scale=-1.0,
                accum_out=zt,
            )
            rz = soft.tile([P, 1], F32, tag=f"rz{bi}")
            nc.vector.reciprocal(out=rz, in_=zt)
            gr = soft.tile([P, 1], F32, tag=f"gr{bi}")
            nc.vector.tensor_mul(gr, rz, gamma_sb)
            Bp = soft.tile([P, C], F16, tag=f"Bp{bi}")
            nc.vector.tensor_scalar_mul(Bp, bt, gr)  # fp16: gamma*att rows
            for bj in range(NB):
                ptw = psum_i.tile([P, P], F16, tag="tp16")
                nc.tensor.transpose(ptw, Bp[:, bj * P : (bj + 1) * P], ident16)
                nc.scalar.copy(out=W[bj][:, bi * P : (bi + 1) * P], in_=ptw)
        for bj in range(NB):
            nc.vector.tensor_add(
                W[bj][:, bj * P : (bj + 1) * P], W[bj][:, bj * P : (bj + 1) * P], ident16
            )

        psum_i_ctx.__exit__(None, None, None)
        psum = ctx.enter_context(tc.tile_pool(name="psum", bufs=2, space="PSUM"))

        # ---------------- Phase 2: y = x @ W ----------------
        for cb in range(NK // CB):
            c0 = cb * CB
            ops = psum.tile([P, CB, C], F32, tag="ops", bufs=2)
            for u in range(CB):
                r0 = (c0 + u) * P
                for bj in range(NB):
                    nc.tensor.matmul(
                        ops[:, u, :],
                        v_sb[bj][:, r0 : r0 + P],
                        W[bj],
                        start=(bj == 0),
                        stop=(bj == NB - 1),
                    )
            ob = opool.tile([P, CB, C], F32)
            nc.scalar.copy(out=ob, in_=ops)
            nc.sync.dma_start(out=y_v[:, c0 : c0 + CB, :], in_=ob)

    nc.finalize()
    return nc


def _get_nc():
    global _nc_cache
    if _nc_cache is None:
        _nc_cache = _build()
    return _nc_cache


def kernel(x, gamma, _trace=False):
    x = np.ascontiguousarray(np.asarray(x), dtype=np.float32)
    gamma = np.ascontiguousarray(np.asarray(gamma), dtype=np.float32)
    nc = _get_nc()
    in_maps = [
        {"x": np.ascontiguousarray(x[b]), "gamma": gamma} for b in range(B)
    ]
    res = run_bass_kernel_spmd(nc, in_maps, list(range(B)), trace=_trace)
    out = np.stack([r["y"] for r in res.results], axis=0)
    if _trace:
        return out, res
    return out


# revision 4
# speedup vs baseline: 1.7608x; 1.1687x over previous
"""Channel attention (B=8, N=16384, C=512) Trainium2 Bass kernel.

Math (per batch b, with v = x^T [C, N]):
    energy  = v @ v^T                      [C, C]   (gram matrix, symmetric)
    att     = softmax(rowmax(e) - e)       == exp(rowmin(e) - e) / Z  (shift-invariant)
    out     = gamma * (att @ v) + v        [C, N]
    y       = out^T                        [N, C]

Sharding: data-parallel over B — core b computes batch b entirely.

Per-core dataflow (single fp16 pass; the 2e-2 rel-err budget has ~30x slack):
  Phase 1 (stream x in 128-row chunks, half-KB DMA/convert granularity so the
  DMA -> ACT(fp16 round) -> PE pipeline stays full with bufs=3):
    energy += hk-block^T @ hk-rowslice  in PSUM fp32, upper block-triangle only
    v-blocks = PE-transpose(hk blocks) -> PSUM fp16 -> DVE copy -> resident
    SBUF v [C, N] fp16 (no DRAM round-trip for the transposed read).
  Interlude:
    mirror the triangle via PE transposes; rowmin; exp(min - e) with fused
    row-sum (ACT accum_out); W = I + gamma/Z * att^T in fp16 via PE transposes.
    (W folds the softmax normalization, the gamma scale AND the residual.)
  Phase 2:
    y[n-chunk] = v-slice^T @ W  accumulated over 4 channel blocks
    (= x @ (I + gamma*att^T) = gamma*(att@v)^T + x, i.e. the final output),
    2-chunk PSUM groups (bufs=4) so the output DMA stream starts early.
"""

import sys

sys.path.insert(0, "/opt/trn_rl_repo")

from contextlib import ExitStack

import numpy as np

import concourse.bass as bass
import concourse.mybir as mybir
import concourse.tile as tile
from concourse import bacc
from concourse.bass_utils import run_bass_kernel_spmd
from concourse.masks import make_identity

B, N, C = 8, 16384, 512
P = 128
NK = N // P  # 128 row chunks
NB = C // P  # 4 channel blocks
KB = 4  # chunks per phase-1 iteration
F32 = mybir.dt.float32
F16 = mybir.dt.float16

_nc_cache = None


def _build():
    nc = bacc.Bacc()
    x_in = nc.dram_tensor("x", [N, C], F32, kind="ExternalInput")
    g_in = nc.dram_tensor("gamma", [1], F32, kind="ExternalInput")
    y_out = nc.dram_tensor("y", [N, C], F32, kind="ExternalOutput")

    with ExitStack() as ctx:
        tc = ctx.enter_context(tile.TileContext(nc))
        const = ctx.enter_context(tc.tile_pool(name="const", bufs=1))
        xpool = ctx.enter_context(tc.tile_pool(name="xpool", bufs=3))
        hpool = ctx.enter_context(tc.tile_pool(name="hpool", bufs=3))
        vpool = ctx.enter_context(tc.tile_pool(name="vpool", bufs=1))
        soft = ctx.enter_context(tc.tile_pool(name="soft", bufs=1))
        opool = ctx.enter_context(tc.tile_pool(name="opool", bufs=4))
        psum_e_ctx = tc.tile_pool(name="psum_e", bufs=1, space="PSUM")
        psum_e = psum_e_ctx.__enter__()
        psum_tp_ctx = tc.tile_pool(name="psum_tp", bufs=2, space="PSUM")
        psum_tp = psum_tp_ctx.__enter__()

        ident16 = const.tile([P, P], F16)
        make_identity(nc, ident16)
        ident32 = const.tile([P, P], F32)
        make_identity(nc, ident32)
        gamma_sb = const.tile([P, 1], F32)

        # [P, NK, C] views: partition = row-within-chunk, mid = chunk index
        x_v = x_in[:].rearrange("(n p) c -> p n c", p=P)
        y_v = y_out[:].rearrange("(n p) c -> p n c", p=P)

        # upper-triangle energy accumulators: row-block bi holds cols [bi*P, C)
        e_ps = [psum_e.tile([P, C - bi * P], F32, name=f"e{bi}", tag=f"e{bi}", bufs=1) for bi in range(NB)]
        # resident transposed x [C, N] fp16 as 4 partition-blocks
        v_sb = [vpool.tile([P, N], F16, name=f"v{bj}", tag=f"v{bj}") for bj in range(NB)]

        # ---------------- Phase 1: energy + transposed copy ----------------
        for kb in range(NK // KB):
            k0 = kb * KB
            xk = xpool.tile([P, KB, C], F32)
            hk = hpool.tile([P, KB, C], F16)
            # half-granularity DMA + fp16 rounding: finer pipeline stages
            nc.sync.dma_start(out=xk[:, 0:2, :], in_=x_v[:, k0 : k0 + 2, :])
            nc.scalar.copy(out=hk[:, 0:2, :], in_=xk[:, 0:2, :])
            nc.sync.dma_start(out=xk[:, 2:4, :], in_=x_v[:, k0 + 2 : k0 + 4, :])
            nc.scalar.copy(out=hk[:, 2:4, :], in_=xk[:, 2:4, :])
            if kb == 0:
                # gamma is only needed at the interlude; keep its 128 tiny
                # descriptors off the x-in queue head (scalar ring instead)
                nc.scalar.dma_start(out=gamma_sb, in_=g_in[:].to_broadcast([P, 1]))
            pt = psum_tp.tile([P, KB, C], F16, tag="pt", bufs=2)
            for u in range(KB):
                k = k0 + u
                first = k == 0
                last = k == NK - 1
                for bi in range(NB):
                    j0 = bi * P
                    nc.tensor.matmul(
                        e_ps[bi], hk[:, u, j0 : j0 + P], hk[:, u, j0:C], start=first, stop=last
                    )
                for bj in range(NB):
                    nc.tensor.transpose(
                        pt[:, u, bj * P : (bj + 1) * P], hk[:, u, bj * P : (bj + 1) * P], ident16
                    )
            for bj in range(NB):
                nc.vector.tensor_copy(
                    out=v_sb[bj][:, k0 * P : (k0 + KB) * P], in_=pt[:, :, bj * P : (bj + 1) * P]
                )

        # ---------------- Interlude: softmax -> W = I + gamma * att^T ----------------
        e_row = [soft.tile([P, C], F32, name=f"erow{bi}", tag=f"erow{bi}") for bi in range(NB)]
        nc.scalar.copy(out=e_row[0], in_=e_ps[0])
        for bi in range(1, NB):
            nc.vector.tensor_copy(out=e_row[bi][:, bi * P : C], in_=e_ps[bi])
        psum_tp_ctx.__exit__(None, None, None)
        psum_e_ctx.__exit__(None, None, None)
        psum_i_ctx = tc.tile_pool(name="psum_i", bufs=2, space="PSUM")
        psum_i = psum_i_ctx.__enter__()
        # mirror the strict-lower blocks from the stored upper triangle
        for bi in range(NB):
            for bj in range(bi):
                ptm = psum_i.tile([P, P], F32, tag="tp")
                nc.tensor.transpose(ptm, e_row[bj][:, bi * P : (bi + 1) * P], ident32)
                nc.vector.tensor_copy(out=e_row[bi][:, bj * P : (bj + 1) * P], in_=ptm)

        W = [soft.tile([P, C], F16, name=f"W{bj}", tag=f"W{bj}") for bj in range(NB)]
        for bi in range(NB):
            mn = soft.tile([P, 1], F32, tag=f"mn{bi}")
            nc.vector.tensor_reduce(
                out=mn, in_=e_row[bi], axis=mybir.AxisListType.X, op=mybir.AluOpType.min
            )
            bt = soft.tile([P, C], F32, tag=f"bt{bi}")
            zt = soft.tile([P, 1], F32, tag=f"zt{bi}")
            nc.scalar.activation(
                out=bt,
                in_=e_row[bi],
                func=mybir.ActivationFunctionType.Exp,
                bias=mn,
                scale=-1.0,
                accum_out=zt,
            )
            rz = soft.tile([P, 1], F32, tag=f"rz{bi}")
            nc.vector.reciprocal(out=rz, in_=zt)
            gr = soft.tile([P, 1], F32, tag=f"gr{bi}")
            nc.vector.tensor_mul(gr, rz, gamma_sb)
            Bp = soft.tile([P, C], F16, tag=f"Bp{bi}")
            nc.vector.tensor_scalar_mul(Bp, bt, gr)  # fp16: gamma*att rows
            for bj in range(NB):
                ptw = psum_i.tile([P, P], F16, tag="tp16")
                nc.tensor.transpose(ptw, Bp[:, bj * P : (bj + 1) * P], ident16)
                nc.scalar.copy(out=W[bj][:, bi * P : (bi + 1) * P], in_=ptw)
        for bj in range(NB):
            nc.vector.tensor_add(
                W[bj][:, bj * P : (bj + 1) * P], W[bj][:, bj * P : (bj + 1) * P], ident16
            )

        psum_i_ctx.__exit__(None, None, None)
        psum = ctx.enter_context(tc.tile_pool(name="psum", bufs=4, space="PSUM"))

        # ---------------- Phase 2: y = x @ W ----------------
        for g in range(NK // 2):
            c0 = g * 2
            ops = psum.tile([P, 2, C], F32, tag="ops", bufs=4)
            for u in range(2):
                r0 = (c0 + u) * P
                for bj in range(NB):
                    nc.tensor.matmul(
                        ops[:, u, :],
                        v_sb[bj][:, r0 : r0 + P],
                        W[bj],
                        start=(bj == 0),
                        stop=(bj == NB - 1),
                    )
            ob = opool.tile([P, 2, C], F32)
            nc.scalar.copy(out=ob, in_=ops)
            nc.sync.dma_start(out=y_v[:, c0 : c0 + 2, :], in_=ob)

    nc.finalize()
    return nc


def _get_nc():
    global _nc_cache
    if _nc_cache is None:
        _nc_cache = _build()
    return _nc_cache


def kernel(x, gamma, _trace=False):
    x = np.ascontiguousarray(np.asarray(x), dtype=np.float32)
    gamma = np.ascontiguousarray(np.asarray(gamma), dtype=np.float32)
    nc = _get_nc()
    in_maps = [
        {"x": np.ascontiguousarray(x[b]), "gamma": gamma} for b in range(B)
    ]
    res = run_bass_kernel_spmd(nc, in_maps, list(range(B)), trace=_trace)
    out = np.stack([r["y"] for r in res.results], axis=0)
    if _trace:
        return out, res
    return out


# revision 7
# speedup vs baseline: 1.7742x; 1.0076x over previous
"""Channel attention (B=8, N=16384, C=512) Trainium2 Bass kernel.

Math (per batch b, with v = x^T [C, N]):
    energy  = v @ v^T                      [C, C]   (gram matrix, symmetric)
    att     = softmax(rowmax(e) - e)       == exp(rowmin(e) - e) / Z  (shift-invariant)
    out     = gamma * (att @ v) + v        [C, N]
    y       = out^T                        [N, C]

Sharding: data-parallel over B — core b computes batch b entirely.

Per-core dataflow (single fp16 pass; the 2e-2 rel-err budget has ~30x slack):
  Phase 1 (stream x in 128-row chunks, half-KB DMA/convert granularity so the
  DMA -> ACT(fp16 round) -> PE pipeline stays full with bufs=3):
    energy += hk-block^T @ hk-rowslice  in PSUM fp32, upper block-triangle only
    v-blocks = PE-transpose(hk blocks) -> PSUM fp16 -> DVE copy -> resident
    SBUF v [C, N] fp16 (no DRAM round-trip for the transposed read).
  Interlude:
    mirror the triangle via PE transposes; rowmin; exp(min - e) with fused
    row-sum (ACT accum_out); W = I + gamma/Z * att^T in fp16 via PE transposes.
    (W folds the softmax normalization, the gamma scale AND the residual.)
  Phase 2:
    y[n-chunk] = v-slice^T @ W  accumulated over 4 channel blocks
    (= x @ (I + gamma*att^T) = gamma*(att@v)^T + x, i.e. the final output),
    2-chunk PSUM groups (bufs=4) so the output DMA stream starts early.
"""

import sys

sys.path.insert(0, "/opt/trn_rl_repo")

from contextlib import ExitStack

import numpy as np

import concourse.bass as bass
import concourse.mybir as mybir
import concourse.tile as tile
from concourse import bacc
from concourse.bass_utils import run_bass_kernel_spmd
from concourse.masks import make_identity

B, N, C = 8, 16384, 512
P = 128
NK = N // P  # 128 row chunks
NB = C // P  # 4 channel blocks
KB = 4  # chunks per phase-1 iteration
F32 = mybir.dt.float32
F16 = mybir.dt.float16

_nc_cache = None


def _build():
    nc = bacc.Bacc()
    x_in = nc.dram_tensor("x", [N, C], F32, kind="ExternalInput")
    g_in = nc.dram_tensor("gamma", [1], F32, kind="ExternalInput")
    y_out = nc.dram_tensor("y", [N, C], F32, kind="ExternalOutput")

    with ExitStack() as ctx:
        tc = ctx.enter_context(tile.TileContext(nc))
        const = ctx.enter_context(tc.tile_pool(name="const", bufs=1))
        xpool = ctx.enter_context(tc.tile_pool(name="xpool", bufs=3))
        hpool = ctx.enter_context(tc.tile_pool(name="hpool", bufs=3))
        vpool = ctx.enter_context(tc.tile_pool(name="vpool", bufs=1))
        soft = ctx.enter_context(tc.tile_pool(name="soft", bufs=1))
        opool = ctx.enter_context(tc.tile_pool(name="opool", bufs=4))
        psum_e_ctx = tc.tile_pool(name="psum_e", bufs=1, space="PSUM")
        psum_e = psum_e_ctx.__enter__()
        psum_tp_ctx = tc.tile_pool(name="psum_tp", bufs=2, space="PSUM")
        psum_tp = psum_tp_ctx.__enter__()

        ident16 = const.tile([P, P], F16)
        make_identity(nc, ident16)
        ident32 = const.tile([P, P], F32)
        make_identity(nc, ident32)
        gamma_sb = const.tile([P, 1], F32)

        # [P, NK, C] views: partition = row-within-chunk, mid = chunk index
        x_v = x_in[:].rearrange("(n p) c -> p n c", p=P)
        y_v = y_out[:].rearrange("(n p) c -> p n c", p=P)

        # upper-triangle energy accumulators: row-block bi holds cols [bi*P, C)
        e_ps = [psum_e.tile([P, C - bi * P], F32, name=f"e{bi}", tag=f"e{bi}", bufs=1) for bi in range(NB)]
        # resident transposed x [C, N] fp16 as 4 partition-blocks
        v_sb = [vpool.tile([P, N], F16, name=f"v{bj}", tag=f"v{bj}") for bj in range(NB)]

        # ---------------- Phase 1: energy + transposed copy ----------------
        for kb in range(NK // KB):
            k0 = kb * KB
            xk = xpool.tile([P, KB, C], F32)
            hk = hpool.tile([P, KB, C], F16)
            # half-granularity DMA + fp16 rounding: finer pipeline stages
            nc.sync.dma_start(out=xk[:, 0:2, :], in_=x_v[:, k0 : k0 + 2, :])
            nc.scalar.copy(out=hk[:, 0:2, :], in_=xk[:, 0:2, :])
            nc.sync.dma_start(out=xk[:, 2:4, :], in_=x_v[:, k0 + 2 : k0 + 4, :])
            nc.scalar.copy(out=hk[:, 2:4, :], in_=xk[:, 2:4, :])
            if kb == 0:
                # gamma is only needed at the interlude; keep its 128 tiny
                # descriptors off the x-in queue head (scalar ring instead)
                nc.scalar.dma_start(out=gamma_sb, in_=g_in[:].to_broadcast([P, 1]))
            pt = psum_tp.tile([P, KB, C], F16, tag="pt", bufs=2)
            for u in range(KB):
                k = k0 + u
                first = k == 0
                last = k == NK - 1
                for bi in range(NB):
                    j0 = bi * P
                    nc.tensor.matmul(
                        e_ps[bi], hk[:, u, j0 : j0 + P], hk[:, u, j0:C], start=first, stop=last
                    )
                for bj in range(NB):
                    nc.tensor.transpose(
                        pt[:, u, bj * P : (bj + 1) * P], hk[:, u, bj * P : (bj + 1) * P], ident16
                    )
            for bj in range(NB):
                nc.vector.tensor_copy(
                    out=v_sb[bj][:, k0 * P : (k0 + KB) * P], in_=pt[:, :, bj * P : (bj + 1) * P]
                )

        # ---------------- Interlude: softmax -> W = I + gamma * att^T ----------------
        e_row = [soft.tile([P, C], F32, name=f"erow{bi}", tag=f"erow{bi}") for bi in range(NB)]
        nc.scalar.copy(out=e_row[0], in_=e_ps[0])
        for bi in range(1, NB):
            nc.vector.tensor_copy(out=e_row[bi][:, bi * P : C], in_=e_ps[bi])
        psum_tp_ctx.__exit__(None, None, None)
        psum_e_ctx.__exit__(None, None, None)
        psum_i_ctx = tc.tile_pool(name="psum_i", bufs=2, space="PSUM")
        psum_i = psum_i_ctx.__enter__()
        # mirror the strict-lower blocks from the stored upper triangle
        for bi in range(NB):
            for bj in range(bi):
                ptm = psum_i.tile([P, P], F32, tag="tp")
                nc.tensor.transpose(ptm, e_row[bj][:, bi * P : (bi + 1) * P], ident32)
                nc.vector.tensor_copy(out=e_row[bi][:, bj * P : (bj + 1) * P], in_=ptm)

        W = [soft.tile([P, C], F16, name=f"W{bj}", tag=f"W{bj}") for bj in range(NB)]
        for bi in range(NB):
            mn = soft.tile([P, 1], F32, tag=f"mn{bi}")
            nc.vector.tensor_reduce(
                out=mn, in_=e_row[bi], axis=mybir.AxisListType.X, op=mybir.AluOpType.min
            )
            bt = soft.tile([P, C], F32, tag=f"bt{bi}")
            zt = soft.tile([P, 1], F32, tag=f"zt{bi}")
            nc.scalar.activation(
                out=bt,
                in_=e_row[bi],
                func=mybir.ActivationFunctionType.Exp,
                bias=mn,
                scale=-1.0,
                accum_out=zt,
            )
            rz = soft.tile([P, 1], F32, tag=f"rz{bi}")
            nc.vector.reciprocal(out=rz, in_=zt)
            gr = soft.tile([P, 1], F32, tag=f"gr{bi}")
            nc.vector.tensor_mul(gr, rz, gamma_sb)
            Bp = soft.tile([P, C], F16, tag=f"Bp{bi}")
            nc.vector.tensor_scalar_mul(Bp, bt, gr)  # fp16: gamma*att rows
            # fold the output-residual identity in BEFORE transposing:
            # T(Bp_diag + I) == T(Bp_diag) + I, so no post-assembly W add needed
            nc.vector.tensor_add(
                Bp[:, bi * P : (bi + 1) * P], Bp[:, bi * P : (bi + 1) * P], ident16
            )
            for bj in range(NB):
                ptw = psum_i.tile([P, P], F16, tag="tp16")
                nc.tensor.transpose(ptw, Bp[:, bj * P : (bj + 1) * P], ident16)
                # split the PSUM->SBUF drains across ACT and DVE so neither
                # serializes the W-assembly tail
                eng = nc.scalar.copy if bj % 2 == 0 else nc.vector.tensor_copy
                eng(out=W[bj][:, bi * P : (bi + 1) * P], in_=ptw)

        psum_i_ctx.__exit__(None, None, None)
        psum = ctx.enter_context(tc.tile_pool(name="psum", bufs=4, space="PSUM"))

        # ---------------- Phase 2: y = x @ W ----------------
        # 2-chunk groups, then single-chunk granularity for the last 4 chunks
        # so the final copy+DMA tail after the last matmul is minimal
        groups = [(c, 2) for c in range(0, NK - 4, 2)] + [(c, 1) for c in range(NK - 4, NK)]
        for c0, gw in groups:
            ops = psum.tile([P, 2, C], F32, tag="ops", bufs=4)
            for u in range(gw):
                r0 = (c0 + u) * P
                for bj in range(NB):
                    nc.tensor.matmul(
                        ops[:, u, :],
                        v_sb[bj][:, r0 : r0 + P],
                        W[bj],
                        start=(bj == 0),
                        stop=(bj == NB - 1),
                    )
            ob = opool.tile([P, 2, C], F32)
            nc.scalar.copy(out=ob[:, 0:gw, :], in_=ops[:, 0:gw, :])
            nc.sync.dma_start(out=y_v[:, c0 : c0 + gw, :], in_=ob[:, 0:gw, :])

    nc.finalize()
    return nc


def _get_nc():
    global _nc_cache
    if _nc_cache is None:
        _nc_cache = _build()
    return _nc_cache


def kernel(x, gamma, _trace=False):
    x = np.ascontiguousarray(np.asarray(x), dtype=np.float32)
    gamma = np.ascontiguousarray(np.asarray(gamma), dtype=np.float32)
    nc = _get_nc()
    in_maps = [
        {"x": np.ascontiguousarray(x[b]), "gamma": gamma} for b in range(B)
    ]
    res = run_bass_kernel_spmd(nc, in_maps, list(range(B)), trace=_trace)
    out = np.stack([r["y"] for r in res.results], axis=0)
    if _trace:
        return out, res
    return out


# revision 18
# speedup vs baseline: 1.7823x; 1.0046x over previous
"""Channel attention (B=8, N=16384, C=512) Trainium2 Bass kernel.

Math (per batch b, with v = x^T [C, N]):
    energy  = v @ v^T                      [C, C]   (gram matrix, symmetric)
    att     = softmax(rowmax(e) - e)       == exp(rowmin(e) - e) / Z  (shift-invariant)
    out     = gamma * (att @ v) + v        [C, N]
    y       = out^T                        [N, C]

Sharding: data-parallel over B — core b computes batch b entirely.

Per-core dataflow (single fp16 pass; the 2e-2 rel-err budget has ~30x slack):
  Phase 1 (stream x in 128-row chunks, half-KB DMA/convert granularity so the
  DMA -> ACT(fp16 round) -> PE pipeline stays full with bufs=3):
    energy += hk-block^T @ hk-rowslice  in PSUM fp32, upper block-triangle only
    v-blocks = PE-transpose(hk blocks) -> PSUM fp16 -> DVE copy -> resident
    SBUF v [C, N] fp16 (no DRAM round-trip for the transposed read).
  Interlude:
    mirror the triangle via PE transposes; rowmin; exp(min - e) with fused
    row-sum (ACT accum_out); W = I + gamma/Z * att^T in fp16 via PE transposes.
    (W folds the softmax normalization, the gamma scale AND the residual.)
  Phase 2:
    y[n-chunk] = v-slice^T @ W  accumulated over 4 channel blocks
    (= x @ (I + gamma*att^T) = gamma*(att@v)^T + x, i.e. the final output),
    2-chunk PSUM groups (bufs=4) so the output DMA stream starts early.
"""

import sys

sys.path.insert(0, "/opt/trn_rl_repo")

from contextlib import ExitStack

import numpy as np

import concourse.bass as bass
import concourse.mybir as mybir
import concourse.tile as tile
from concourse import bacc
from concourse.bass_utils import run_bass_kernel_spmd
from concourse.masks import make_identity

B, N, C = 8, 16384, 512
P = 128
NK = N // P  # 128 row chunks
NB = C // P  # 4 channel blocks
KB = 4  # chunks per phase-1 iteration
F32 = mybir.dt.float32
F16 = mybir.dt.float16

_nc_cache = None


def _build():
    nc = bacc.Bacc()
    x_in = nc.dram_tensor("x", [N, C], F32, kind="ExternalInput")
    g_in = nc.dram_tensor("gamma", [1], F32, kind="ExternalInput")
    y_out = nc.dram_tensor("y", [N, C], F32, kind="ExternalOutput")

    with ExitStack() as ctx:
        tc = ctx.enter_context(tile.TileContext(nc))
        const = ctx.enter_context(tc.tile_pool(name="const", bufs=1))
        xpool = ctx.enter_context(tc.tile_pool(name="xpool", bufs=3))
        hpool = ctx.enter_context(tc.tile_pool(name="hpool", bufs=3))
        vpool = ctx.enter_context(tc.tile_pool(name="vpool", bufs=1))
        soft = ctx.enter_context(tc.tile_pool(name="soft", bufs=1))
        opool = ctx.enter_context(tc.tile_pool(name="opool", bufs=4))
        psum_e_ctx = tc.tile_pool(name="psum_e", bufs=1, space="PSUM")
        psum_e = psum_e_ctx.__enter__()
        psum_tp_ctx = tc.tile_pool(name="psum_tp", bufs=2, space="PSUM")
        psum_tp = psum_tp_ctx.__enter__()

        ident16 = const.tile([P, P], F16)
        make_identity(nc, ident16)
        ident32 = const.tile([P, P], F32)
        make_identity(nc, ident32)
        gamma_sb = const.tile([P, 1], F32)

        # [P, NK, C] views: partition = row-within-chunk, mid = chunk index
        x_v = x_in[:].rearrange("(n p) c -> p n c", p=P)
        y_v = y_out[:].rearrange("(n p) c -> p n c", p=P)

        # upper-triangle energy accumulators: row-block bi holds cols [bi*P, C)
        e_ps = [psum_e.tile([P, C - bi * P], F32, name=f"e{bi}", tag=f"e{bi}", bufs=1) for bi in range(NB)]
        # resident transposed x [C, N] fp16 as 4 partition-blocks
        v_sb = [vpool.tile([P, N], F16, name=f"v{bj}", tag=f"v{bj}") for bj in range(NB)]

        # ---------------- Phase 1: energy + transposed copy ----------------
        for kb in range(NK // KB):
            k0 = kb * KB
            xk = xpool.tile([P, KB, C], F32)
            hk = hpool.tile([P, KB, C], F16)
            # half-granularity DMA + fp16 rounding: finer pipeline stages
            nc.sync.dma_start(out=xk[:, 0:2, :], in_=x_v[:, k0 : k0 + 2, :])
            nc.scalar.copy(out=hk[:, 0:2, :], in_=xk[:, 0:2, :])
            nc.sync.dma_start(out=xk[:, 2:4, :], in_=x_v[:, k0 + 2 : k0 + 4, :])
            nc.scalar.copy(out=hk[:, 2:4, :], in_=xk[:, 2:4, :])
            if kb == 0:
                # gamma is only needed at the interlude; keep its 128 tiny
                # descriptors off the x-in queue head (scalar ring instead)
                nc.scalar.dma_start(out=gamma_sb, in_=g_in[:].to_broadcast([P, 1]))
            pt = psum_tp.tile([P, KB, C], F16, tag="pt", bufs=2)
            for u in range(KB):
                k = k0 + u
                first = k == 0
                last = k == NK - 1
                for bi in range(NB):
                    j0 = bi * P
                    nc.tensor.matmul(
                        e_ps[bi], hk[:, u, j0 : j0 + P], hk[:, u, j0:C], start=first, stop=last
                    )
                for bj in range(NB):
                    nc.tensor.transpose(
                        pt[:, u, bj * P : (bj + 1) * P], hk[:, u, bj * P : (bj + 1) * P], ident16
                    )
            for bj in range(NB):
                nc.vector.tensor_copy(
                    out=v_sb[bj][:, k0 * P : (k0 + KB) * P], in_=pt[:, :, bj * P : (bj + 1) * P]
                )

        # ---------------- Interlude: softmax -> W = I + gamma * att^T ----------------
        e_row = [soft.tile([P, C], F32, name=f"erow{bi}", tag=f"erow{bi}") for bi in range(NB)]
        nc.scalar.copy(out=e_row[0], in_=e_ps[0])
        for bi in range(1, NB):
            nc.vector.tensor_copy(out=e_row[bi][:, bi * P : C], in_=e_ps[bi])
        mn = [soft.tile([P, 1], F32, name=f"mn{bi}", tag=f"mn{bi}") for bi in range(NB)]
        # row-block 0 needs no mirrored blocks: hoist its rowmin ahead of the
        # mirror copies in DVE program order (no PSUM dependency either)
        nc.vector.tensor_reduce(
            out=mn[0], in_=e_row[0], axis=mybir.AxisListType.X, op=mybir.AluOpType.min
        )
        psum_tp_ctx.__exit__(None, None, None)
        psum_e_ctx.__exit__(None, None, None)
        psum_i_ctx = tc.tile_pool(name="psum_i", bufs=2, space="PSUM")
        psum_i = psum_i_ctx.__enter__()

        # mirror the strict-lower blocks from the stored upper triangle,
        # ordered so lower-numbered row-blocks complete first
        for bi, bj in [(1, 0), (2, 0), (3, 0), (2, 1), (3, 1), (3, 2)]:
            ptm = psum_i.tile([P, P], F32, tag="tp", bufs=3)
            nc.tensor.transpose(ptm, e_row[bj][:, bi * P : (bi + 1) * P], ident32)
            nc.vector.tensor_copy(out=e_row[bi][:, bj * P : (bj + 1) * P], in_=ptm)

        W = [soft.tile([P, C], F16, name=f"W{bj}", tag=f"W{bj}") for bj in range(NB)]
        for bi in range(NB):
            if bi > 0:
                nc.vector.tensor_reduce(
                    out=mn[bi], in_=e_row[bi], axis=mybir.AxisListType.X, op=mybir.AluOpType.min
                )
            bt = soft.tile([P, C], F32, tag=f"bt{bi}")
            zt = soft.tile([P, 1], F32, tag=f"zt{bi}")
            nc.scalar.activation(
                out=bt,
                in_=e_row[bi],
                func=mybir.ActivationFunctionType.Exp,
                bias=mn[bi],
                scale=-1.0,
                accum_out=zt,
            )
            rz = soft.tile([P, 1], F32, tag=f"rz{bi}")
            nc.vector.reciprocal(out=rz, in_=zt)
            gr = soft.tile([P, 1], F32, tag=f"gr{bi}")
            nc.vector.tensor_mul(gr, rz, gamma_sb)
            Bp = soft.tile([P, C], F16, tag=f"Bp{bi}")
            nc.vector.tensor_scalar_mul(Bp, bt, gr)  # fp16: gamma*att rows
            # fold the output-residual identity in BEFORE transposing:
            # T(Bp_diag + I) == T(Bp_diag) + I, so no post-assembly W add needed
            nc.vector.tensor_add(
                Bp[:, bi * P : (bi + 1) * P], Bp[:, bi * P : (bi + 1) * P], ident16
            )
            for bj in range(NB):
                # 5 rotating bufs (bank-granular PSUM, 8 banks total): enough
                # in-flight transposes that none waits on its copy's drain
                ptw = psum_i.tile([P, P], F16, tag="tp16", bufs=5)
                nc.tensor.transpose(ptw, Bp[:, bj * P : (bj + 1) * P], ident16)
                # split the PSUM->SBUF drains across ACT and DVE so neither
                # serializes the W-assembly tail
                eng = nc.scalar.copy if bj % 2 == 0 else nc.vector.tensor_copy
                eng(out=W[bj][:, bi * P : (bi + 1) * P], in_=ptw)

        psum_i_ctx.__exit__(None, None, None)
        psum = ctx.enter_context(tc.tile_pool(name="psum", bufs=4, space="PSUM"))

        # ---------------- Phase 2: y = x @ W ----------------
        # 2-chunk groups, then single-chunk granularity for the last 4 chunks
        # so the final copy+DMA tail after the last matmul is minimal
        groups = [(c, 2) for c in range(0, NK - 4, 2)] + [(c, 1) for c in range(NK - 4, NK)]
        for c0, gw in groups:
            ops = psum.tile([P, 2, C], F32, tag="ops", bufs=4)
            for u in range(gw):
                r0 = (c0 + u) * P
                for bj in range(NB):
                    nc.tensor.matmul(
                        ops[:, u, :],
                        v_sb[bj][:, r0 : r0 + P],
                        W[bj],
                        start=(bj == 0),
                        stop=(bj == NB - 1),
                    )
            ob = opool.tile([P, 2, C], F32)
            # alternate the drain engine for the single-chunk tail groups so
            # the final copies don't serialize on ACT
            if gw == 1 and (c0 % 2) == 1:
                nc.vector.tensor_copy(out=ob[:, 0:gw, :], in_=ops[:, 0:gw, :])
            else:
                nc.scalar.copy(out=ob[:, 0:gw, :], in_=ops[:, 0:gw, :])
            nc.sync.dma_start(out=y_v[:, c0 : c0 + gw, :], in_=ob[:, 0:gw, :])

    nc.finalize()
    return nc


def _get_nc():
    global _nc_cache
    if _nc_cache is None:
        _nc_cache = _build()
    return _nc_cache


def kernel(x, gamma, _trace=False):
    x = np.ascontiguousarray(np.asarray(x), dtype=np.float32)
    gamma = np.ascontiguousarray(np.asarray(gamma), dtype=np.float32)
    nc = _get_nc()
    in_maps = [
        {"x": np.ascontiguousarray(x[b]), "gamma": gamma} for b in range(B)
    ]
    res = run_bass_kernel_spmd(nc, in_maps, list(range(B)), trace=_trace)
    out = np.stack([r["y"] for r in res.results], axis=0)
    if _trace:
        return out, res
    return out
